# revision 1
# baseline (speedup 1.0000x reference)
"""Head-sharded (tensor-parallel) attention kernel for 8 NeuronCores.

Sharding (per spec hint): 16 q-heads -> 2 per core; 4 kv-heads -> each kv
head serves the core-pair that owns its 4 query heads (cores 2g, 2g+1 get
kv head g, one head each pair-half duplicated).  Wq/Wk/Wv sharded by rows,
Wo by columns, KV cache by kv-head.  Global fake-quant amaxes are resolved
with lax.pmax inside the pmapped kernel; o_proj partials are summed with
lax.psum (the all-reduce after o_proj).
"""

import numpy as np
import jax
import jax.numpy as jnp
from functools import partial
import math

HID, NH, NKV, HD = 2048, 16, 4, 128
S, CLEN = 512, 8192
NCORES = 8
QH_PER_CORE = NH // NCORES  # 2
AX = "x"


def _fq_with_amax(x, amax, bits=8):
    qmax = 2.0 ** (bits - 1) - 1.0
    scale = jnp.maximum(amax, 1e-8) / qmax
    return jnp.clip(jnp.round(x / scale), -qmax - 1.0, qmax) * scale


def _gmax(x):
    """global (cross-core) amax of |x|"""
    return jax.lax.pmax(jnp.max(jnp.abs(x)), AX)


def _fq_rows(W):
    """per-out-channel (row) fake quant, row-local so no collective"""
    qmax = 127.0
    amax = jnp.max(jnp.abs(W), axis=1, keepdims=True)
    scale = jnp.maximum(amax, 1e-8) / qmax
    return jnp.clip(jnp.round(W / scale), -128.0, 127.0) * scale


def _rotate_half(x):
    x1, x2 = jnp.split(x, 2, axis=-1)
    return jnp.concatenate([-x2, x1], axis=-1)


def _core_fn(x, cos, sin, ck, cv, mask, Wq, bq, Wk, bk, Wv, bv, Wo, wo_amax_l):
    # x:[S,HID] replicated; Wq:[256,HID] bq:[256]; Wk/Wv:[128,HID] bk/bv:[128]
    # ck/cv:[CLEN,HD] this core's kv head; mask:[S,CLEN]; Wo:[HID,256] col slice
    # cache const fake-quant (global per-tensor amax)
    ck_q = _fq_with_amax(ck, _gmax(ck))
    cv_q = _fq_with_amax(cv, _gmax(cv))

    # qkv projections (dynamic act quant is per-tensor on replicated x -> local)
    xq = _fq_with_amax(x, jnp.max(jnp.abs(x)))
    q = xq @ _fq_rows(Wq).T + bq          # [S, 256]
    k = xq @ _fq_rows(Wk).T + bk          # [S, 128]
    v = xq @ _fq_rows(Wv).T + bv          # [S, 128]
    q = q.reshape(S, QH_PER_CORE, HD).transpose(1, 0, 2)   # [2,S,HD]

    # RoPE
    q = q * cos[None] + _rotate_half(q) * sin[None]
    k = k * cos + _rotate_half(k) * sin

    # sliding cache update
    k_cache = jnp.concatenate([ck_q[S:], k], axis=0)       # [CLEN, HD]
    v_cache = jnp.concatenate([cv_q[S:], v], axis=0)

    # fq_matmul(q4, kT): per-tensor amaxes are global across all heads
    q_q = _fq_with_amax(q, _gmax(q))
    k_q = _fq_with_amax(k_cache, _gmax(k_cache))
    attn = jnp.einsum("hsd,cd->hsc", q_q, k_q)             # [2,S,CLEN]
    attn = attn / math.sqrt(HD) + mask[None]
    attn = jax.nn.softmax(attn, axis=-1)

    # fq_matmul(attn, cache_v): global amaxes again
    a_q = _fq_with_amax(attn, _gmax(attn))
    v_q = _fq_with_amax(v_cache, _gmax(v_cache))
    out = jnp.einsum("hsc,cd->hsd", a_q, v_q)              # [2,S,HD]

    # o_proj: DynamicQuantLinear(out_flat, Wo).  out_flat per-tensor amax is
    # global; Wo per-row amax spans the full (sharded) input dim -> pmax.
    out_flat = out.transpose(1, 0, 2).reshape(S, QH_PER_CORE * HD)  # [S,256]
    wo_amax = jax.lax.pmax(wo_amax_l, AX)                  # [HID]
    wo_scale = jnp.maximum(wo_amax, 1e-8) / 127.0
    Wo_q = jnp.clip(jnp.round(Wo / wo_scale[:, None]), -128.0, 127.0) * wo_scale[:, None]
    o_amax = _gmax(out_flat)
    o_scale = jnp.maximum(o_amax, 1e-8) / 127.0
    xq2 = jnp.clip(jnp.round(out_flat / o_scale), -128.0, 127.0) * o_scale
    y = jax.lax.psum(xq2 @ Wo_q.T, AX)                     # [S, HID]

    # cache outputs re-fake-quantized (global amax over all kv heads)
    k_out = _fq_with_amax(k, _gmax(k))
    v_out = _fq_with_amax(v, _gmax(v))
    return y, k_out, v_out


_pmapped = None


def _get_pmapped():
    global _pmapped
    if _pmapped is None:
        _pmapped = jax.pmap(_core_fn, axis_name=AX, devices=jax.devices()[:NCORES])
    return _pmapped


def kernel(hidden_states, cos, sin, cache_k, cache_v, mask, Wq, bq, Wk, bk, Wv, bv, Wo):
    f32 = np.float32
    x = np.asarray(hidden_states, f32).reshape(S, HID)
    cosr = np.asarray(cos, f32).reshape(S, HD)
    sinr = np.asarray(sin, f32).reshape(S, HD)
    maskr = np.asarray(mask, f32).reshape(S, CLEN)
    Wq, Wk, Wv, Wo = (np.asarray(a, f32) for a in (Wq, Wk, Wv, Wo))
    bq, bk, bv = (np.asarray(a, f32) for a in (bq, bk, bv))
    ck = np.asarray(cache_k, f32)
    cv = np.asarray(cache_v, f32)

    rep = lambda a: np.broadcast_to(a, (NCORES,) + a.shape)
    # per-core shards
    Wq_s = Wq.reshape(NH, HD, HID).reshape(NCORES, QH_PER_CORE * HD, HID)
    bq_s = bq.reshape(NCORES, QH_PER_CORE * HD)
    kvh = np.arange(NCORES) // 2                       # kv head per core
    Wk_s = Wk.reshape(NKV, HD, HID)[kvh]
    bk_s = bk.reshape(NKV, HD)[kvh]
    Wv_s = Wv.reshape(NKV, HD, HID)[kvh]
    bv_s = bv.reshape(NKV, HD)[kvh]
    ck_s = ck[kvh]
    cv_s = cv[kvh]
    # Wo column shards [HID, 256] per core; per-row amax of the LOCAL slice
    Wo_s = Wo.reshape(HID, NH, HD).reshape(HID, NCORES, QH_PER_CORE * HD).transpose(1, 0, 2)
    wo_amax_l = np.abs(Wo_s).max(axis=2)               # [NCORES, HID]

    y, k_out, v_out = _get_pmapped()(
        rep(x), rep(cosr), rep(sinr), ck_s, cv_s, rep(maskr),
        Wq_s, bq_s, Wk_s, bk_s, Wv_s, bv_s, Wo_s, wo_amax_l,
    )
    y = np.asarray(y[0]).reshape(1, S, HID)
    k_out = np.asarray(k_out[::2])                     # [NKV, S, HD]
    v_out = np.asarray(v_out[::2])
    return (y, k_out, v_out)


# revision 2
# speedup vs baseline: 1.4488x; 1.4488x over previous
"""Head-sharded (tensor-parallel) attention kernel for 8 NeuronCores.

Sharding (per spec hint): 16 q-heads -> 2 per core; kv head g=c//2 per core;
Wq/Wk/Wv sharded by rows, Wo by columns, KV cache by kv-head.  Global
fake-quant amaxes are resolved with ONE batched lax.pmax before attention,
one pmax for the attention-prob amax, one for the o_proj input amax, and a
final lax.psum implements the all-reduce after o_proj.
"""

import numpy as np
import jax
import jax.numpy as jnp
from functools import partial
import math

HID, NH, NKV, HD = 2048, 16, 4, 128
S, CLEN = 512, 8192
NCORES = 8
QH = NH // NCORES  # q heads per core
AX = "x"


def _fq_amax(x, amax, bits=8):
    qmax = 2.0 ** (bits - 1) - 1.0
    scale = jnp.maximum(amax, 1e-8) / qmax
    return jnp.clip(jnp.round(x / scale), -qmax - 1.0, qmax) * scale


def _q_scalar(a, scale):
    # quantized value of the max-abs element (fq is monotone in |x|)
    return jnp.clip(jnp.round(a / scale), -128.0, 127.0) * scale


def _fq_rows(W):
    amax = jnp.max(jnp.abs(W), axis=1, keepdims=True)
    scale = jnp.maximum(amax, 1e-8) / 127.0
    return jnp.clip(jnp.round(W / scale), -128.0, 127.0) * scale


def _rot(x):
    x1, x2 = jnp.split(x, 2, axis=-1)
    return jnp.concatenate([-x2, x1], axis=-1)


def _core_fn(use_mask, x, cos, sin, ck, cv, mask, Wq, bq, Wk, bk, Wv, bv, Wo, woam):
    # projections on replicated x (per-tensor act quant is local)
    xq = _fq_amax(x, jnp.max(jnp.abs(x)))
    q = xq @ _fq_rows(Wq).T + bq                     # [S, 256]
    k = xq @ _fq_rows(Wk).T + bk                     # [S, HD]
    v = xq @ _fq_rows(Wv).T + bv                     # [S, HD]
    q = q.reshape(S, QH, HD).transpose(1, 0, 2)      # [QH,S,HD]
    q = q * cos[None] + _rot(q) * sin[None]
    k = k * cos + _rot(k) * sin

    # ONE batched collective for every pre-attention global amax
    loc = jnp.concatenate([
        jnp.stack([
            jnp.max(jnp.abs(ck)),            # 0 full old k cache
            jnp.max(jnp.abs(ck[S:])),        # 1 kept old k cache
            jnp.max(jnp.abs(cv)),            # 2 full old v cache
            jnp.max(jnp.abs(cv[S:])),        # 3 kept old v cache
            jnp.max(jnp.abs(q)),             # 4 q (roped)
            jnp.max(jnp.abs(k)),             # 5 new k (roped)
            jnp.max(jnp.abs(v)),             # 6 new v
        ]), woam])                           # 7: per-row |Wo| over local cols
    g = jax.lax.pmax(loc, AX)
    s_ck = jnp.maximum(g[0], 1e-8) / 127.0
    s_cv = jnp.maximum(g[2], 1e-8) / 127.0
    am_kc = jnp.maximum(_q_scalar(g[1], s_ck), g[5])   # amax of concat k cache
    am_vc = jnp.maximum(_q_scalar(g[3], s_cv), g[6])
    wo_scale = jnp.maximum(g[7:], 1e-8) / 127.0

    k_cache = jnp.concatenate([_fq_amax(ck[S:], g[0]), k], axis=0)   # [CLEN,HD]
    v_cache = jnp.concatenate([_fq_amax(cv[S:], g[2]), v], axis=0)

    q_q = _fq_amax(q, g[4])
    k_q = _fq_amax(k_cache, am_kc)
    attn = jnp.einsum("hsd,cd->hsc", q_q, k_q) / math.sqrt(HD)
    if use_mask:
        attn = attn + mask[None]
    attn = jax.nn.softmax(attn, axis=-1)

    a_q = _fq_amax(attn, jax.lax.pmax(jnp.max(attn), AX))
    v_q = _fq_amax(v_cache, am_vc)
    out = jnp.einsum("hsc,cd->hsd", a_q, v_q)        # [QH,S,HD]

    out_flat = out.transpose(1, 0, 2).reshape(S, QH * HD)
    Wo_q = jnp.clip(jnp.round(Wo / wo_scale[:, None]), -128.0, 127.0) * wo_scale[:, None]
    xq2 = _fq_amax(out_flat, jax.lax.pmax(jnp.max(jnp.abs(out_flat)), AX))
    y = jax.lax.psum(xq2 @ Wo_q.T, AX)               # all-reduce after o_proj

    k_out = _fq_amax(k, g[5])
    v_out = _fq_amax(v, g[6])
    return y, k_out, v_out


_pmapped = {}


def _get(use_mask):
    if use_mask not in _pmapped:
        _pmapped[use_mask] = jax.pmap(
            partial(_core_fn, use_mask),
            axis_name=AX, devices=jax.devices()[:NCORES])
    return _pmapped[use_mask]


def kernel(hidden_states, cos, sin, cache_k, cache_v, mask, Wq, bq, Wk, bk, Wv, bv, Wo):
    f32 = np.float32
    x = np.asarray(hidden_states, f32).reshape(S, HID)
    cosr = np.asarray(cos, f32).reshape(S, HD)
    sinr = np.asarray(sin, f32).reshape(S, HD)
    maskr = np.asarray(mask, f32).reshape(S, CLEN)
    use_mask = bool(maskr.any())
    Wq, Wk, Wv, Wo = (np.asarray(a, f32) for a in (Wq, Wk, Wv, Wo))
    bq, bk, bv = (np.asarray(a, f32) for a in (bq, bk, bv))
    ck = np.asarray(cache_k, f32)
    cv = np.asarray(cache_v, f32)

    rep = lambda a: np.broadcast_to(a, (NCORES,) + a.shape)
    Wq_s = Wq.reshape(NCORES, QH * HD, HID)
    bq_s = bq.reshape(NCORES, QH * HD)
    kvh = np.arange(NCORES) // 2
    Wk_s = Wk.reshape(NKV, HD, HID)[kvh]
    bk_s = bk.reshape(NKV, HD)[kvh]
    Wv_s = Wv.reshape(NKV, HD, HID)[kvh]
    bv_s = bv.reshape(NKV, HD)[kvh]
    ck_s = ck[kvh]
    cv_s = cv[kvh]
    Wo_s = Wo.reshape(HID, NCORES, QH * HD).transpose(1, 0, 2)  # [8,HID,256]
    woam = np.abs(Wo_s).max(axis=2)                             # [8,HID]

    mask_arg = rep(maskr) if use_mask else rep(np.zeros((1, 1), f32))
    y, k_out, v_out = _get(use_mask)(
        rep(x), rep(cosr), rep(sinr), ck_s, cv_s, mask_arg,
        Wq_s, bq_s, Wk_s, bk_s, Wv_s, bv_s, Wo_s, woam)
    y = np.asarray(y[0]).reshape(1, S, HID)
    return (y, np.asarray(k_out[::2]), np.asarray(v_out[::2]))
